# revision 4
# baseline (speedup 1.0000x reference)
"""NonLocal1D block on 8 trn2 NeuronCores — batch-parallel, BN via AllReduce.

Problem (per batch b):
    theta = theta_w @ x + theta_b          [Ci, L]
    phi   = maxpool2(phi_w @ x + phi_b)    [Ci, L/2]
    g     = maxpool2(g_w @ x + g_b)        [Ci, L/2]
    f     = softmax_k(theta^T phi)         [L, L/2]
    y     = g @ f^T                        [Ci, L]
    z     = wz_w @ y + wz_b                [C, L]
    out   = BN_train(z) * gamma + beta + x (BN stats over batch+length)

Device strategy (1 batch element per core):
  - all matmuls in fp32r (full PE rate, ~2^-13 operand rounding)
  - attention computed transposed: fT[k, q] = phi^T theta, so that
    * exp(fT) tiles feed the second matmul directly (contraction over k
      = partition dim), no transposes anywhere
    * sum_k exp(fT) is a ones-vector matmul
  - softmax without max-subtraction: |f| <= ~31 on these inputs, exp
    fits fp32 easily and softmax is shift-invariant so accuracy matches
  - phi bias dropped (adds a per-query constant to f -> softmax invariant)
  - g bias folded into wz_b on the host: wz_b' = wz_b + wz_w @ g_b
    (exact because softmax rows sum to 1)
  - maxpool of g is done in transposed layout by computing gT on even and
    odd positions separately (strided stationary operand) and taking the
    elementwise max
  - x is streamed (never fully resident): phase A consumes column chunks
    for phi/gT, phase B re-streams them for theta, phase D re-streams for
    the residual
  - BN batch stats: per-core sum / sum-of-squares per channel, AllReduce
    across the 8 cores, normalization + residual applied on-device
"""
import sys

if "/opt/trn_rl_repo" not in sys.path:
    sys.path.insert(0, "/opt/trn_rl_repo")

import numpy as np

import concourse.bass as bass
import concourse.tile as tile
from concourse import bacc, mybir
from concourse.bass_utils import run_bass_kernel_spmd

N_CORES = 8
B, C, L = 8, 512, 4096
CI = C // 2            # 256
L2 = L // 2            # 2048
QB = 512               # query block (free dim of most matmuls)
NQB = L // QB          # 8
NKT = L2 // 128        # 16 k-tiles
NCT = C // 128         # 4 c-tiles
NIT = CI // 128        # 2 ci-tiles
FP32 = mybir.dt.float32
FP32R = mybir.dt.float32r
ID = mybir.ActivationFunctionType.Identity
EXP = mybir.ActivationFunctionType.Exp

_CACHE = {}


def _load_x_chunk(nc, stage, xc, x_d, ls):
    """DMA x[:, ls:ls+QB] (all C rows) and round into fp32r chunk xc."""
    for ct in range(NCT):
        xf = stage.tile([128, QB], FP32, tag="xf")
        nc.sync.dma_start(xf[:], x_d[ct * 128:(ct + 1) * 128, ls:ls + QB])
        nc.vector.tensor_copy(xc[:, ct, :], xf[:])


def _emit_mm2(nc, gT, ones_r, yps, s_ps, kt, e_t, nkt):
    for it in range(NIT):
        nc.tensor.matmul(
            yps[it][:], gT[:, kt, it * 128:(it + 1) * 128].opt(), e_t[:],
            start=(kt == 0), stop=(kt == nkt - 1), skip_group_check=True)
    nc.tensor.matmul(s_ps[:], ones_r[:], e_t[:],
                     start=(kt == 0), stop=(kt == nkt - 1),
                     skip_group_check=True)


def build_nc():
    nc = bacc.Bacc("TRN2", target_bir_lowering=False, debug=False,
                   num_devices=N_CORES)
    x_d = nc.dram_tensor("x", [C, L], FP32, kind="ExternalInput")
    thw_d = nc.dram_tensor("thw_t", [C, CI], FP32, kind="ExternalInput")
    phw_d = nc.dram_tensor("phw_t", [C, CI], FP32, kind="ExternalInput")
    gw_d = nc.dram_tensor("gw_t", [C, CI], FP32, kind="ExternalInput")
    wzw_d = nc.dram_tensor("wzw_t", [CI, C], FP32, kind="ExternalInput")
    tb_d = nc.dram_tensor("tb", [CI, 1], FP32, kind="ExternalInput")
    wzb_d = nc.dram_tensor("wzb", [C, 1], FP32, kind="ExternalInput")
    gam_d = nc.dram_tensor("gam", [C, 1], FP32, kind="ExternalInput")
    bet_d = nc.dram_tensor("bet", [C, 1], FP32, kind="ExternalInput")
    out_d = nc.dram_tensor("out", [C, L], FP32, kind="ExternalOutput")

    with tile.TileContext(nc) as tc:
        with (
            tc.tile_pool(name="persist", bufs=1) as pers,
            tc.tile_pool(name="xchunk", bufs=2) as xchunk,
            tc.tile_pool(name="stage", bufs=4) as stage,
            tc.tile_pool(name="stage2", bufs=2) as stage2,
            tc.tile_pool(name="pooltmp", bufs=2) as pooltmp,
            tc.tile_pool(name="epool", bufs=3) as epool,
            tc.tile_pool(name="thpool", bufs=2) as thpool,
            tc.tile_pool(name="ypool", bufs=2) as ypool,
            tc.tile_pool(name="small", bufs=1) as small,
            tc.tile_pool(name="ps_ft", bufs=2, space="PSUM") as ps_ft,
            tc.tile_pool(name="ps_y", bufs=2, space="PSUM") as ps_y,
            tc.tile_pool(name="ps_s", bufs=1, space="PSUM") as ps_s,
            tc.tile_pool(name="ps_bc", bufs=1, space="PSUM") as ps_bc,
            tc.tile_pool(name="ps_z", bufs=2, space="PSUM") as ps_z,
            tc.tile_pool(name="dram", bufs=1, space="DRAM") as dram,
        ):
            # ---- weights + constants into SBUF (fp32 stage -> fp32r) ----
            thw = pers.tile([128, NCT, CI], FP32R, tag="thw")
            phw = pers.tile([128, NCT, CI], FP32R, tag="phw")
            gw = pers.tile([128, NCT, CI], FP32R, tag="gw")
            for w_d, w_r in ((thw_d, thw), (phw_d, phw), (gw_d, gw)):
                ws = stage2.tile([128, NCT * CI], FP32, tag="wstage")
                for ct in range(NCT):
                    nc.sync.dma_start(ws[:, ct * CI:(ct + 1) * CI],
                                      w_d[ct * 128:(ct + 1) * 128, :])
                for ct in range(NCT):
                    nc.vector.tensor_copy(w_r[:, ct, :],
                                          ws[:, ct * CI:(ct + 1) * CI])
            wzw = pers.tile([128, NIT, C], FP32R, tag="wzw")
            ws = stage2.tile([128, NIT * C], FP32, tag="wstage")
            for it in range(NIT):
                nc.sync.dma_start(ws[:, it * C:(it + 1) * C],
                                  wzw_d[it * 128:(it + 1) * 128, :])
            for it in range(NIT):
                nc.vector.tensor_copy(wzw[:, it, :], ws[:, it * C:(it + 1) * C])

            tb = pers.tile([128, NIT], FP32, tag="tb")
            for it in range(NIT):
                nc.sync.dma_start(tb[:, it:it + 1], tb_d[it * 128:(it + 1) * 128, :])
            wzb = pers.tile([128, NCT], FP32, tag="wzb")
            gam = pers.tile([128, NCT], FP32, tag="gam")
            bet = pers.tile([128, NCT], FP32, tag="bet")
            for v_d, v in ((wzb_d, wzb), (gam_d, gam), (bet_d, bet)):
                for ct in range(NCT):
                    nc.sync.dma_start(v[:, ct:ct + 1],
                                      v_d[ct * 128:(ct + 1) * 128, :])

            onesf = pers.tile([128, 1], FP32, tag="onesf")
            nc.vector.memset(onesf[:], 1.0)
            ones_r = pers.tile([128, 1], FP32R, tag="ones_r")
            nc.vector.tensor_copy(ones_r[:], onesf[:])
            ones1f = pers.tile([1, 128], FP32, tag="ones1f")
            nc.vector.memset(ones1f[:], 1.0)
            ones1_r = pers.tile([1, 128], FP32R, tag="ones1_r")
            nc.vector.tensor_copy(ones1_r[:], ones1f[:])

            # ---- persistent activations ----
            phi = pers.tile([128, NIT, L2], FP32R, tag="phi")     # [ci, k]
            gT = pers.tile([128, NKT, CI], FP32R, tag="gT")       # [k, ci]

            # ============ phase A: stream x -> phi + gT ==================
            for lb in range(NQB):
                ls = lb * QB
                xc = xchunk.tile([128, NCT, QB], FP32R, tag="xc")
                _load_x_chunk(nc, stage, xc, x_d, ls)
                # phi: [ci, l] chunk -> maxpool2 along l -> [ci, k] (no bias)
                for it in range(NIT):
                    pp = ps_ft.tile([128, QB], FP32, tag="ft")
                    for ct in range(NCT):
                        nc.tensor.matmul(
                            pp[:], phw[:, ct, it * 128:(it + 1) * 128].opt(),
                            xc[:, ct, :].opt(),
                            start=(ct == 0), stop=(ct == NCT - 1))
                    ptmp = pooltmp.tile([128, QB // 2], FP32, tag="ptmp")
                    nc.vector.tensor_copy(ptmp[:], pp[:, 0:QB:2])
                    nc.vector.tensor_max(
                        phi[:, it, ls // 2:ls // 2 + QB // 2].opt(),
                        ptmp[:], pp[:, 1:QB:2])
                # gT: [k, ci] tiles for the two k-tiles in this l-chunk
                # (no bias; folded into wz_b on host)
                for sub in range(2):
                    kt = 2 * lb + sub
                    off = sub * 256
                    pe = ps_ft.tile([128, CI], FP32, tag="ft")
                    po = ps_ft.tile([128, CI], FP32, tag="ft")
                    for ct in range(NCT):
                        nc.tensor.matmul(
                            pe[:], xc[:, ct, off:off + 256:2].opt(),
                            gw[:, ct, :].opt(),
                            start=(ct == 0), stop=(ct == NCT - 1))
                    for ct in range(NCT):
                        nc.tensor.matmul(
                            po[:], xc[:, ct, off + 1:off + 256:2].opt(),
                            gw[:, ct, :].opt(),
                            start=(ct == 0), stop=(ct == NCT - 1))
                    gtmp = pooltmp.tile([128, CI], FP32, tag="gtmp")
                    nc.vector.tensor_copy(gtmp[:], pe[:])
                    nc.vector.tensor_max(gT[:, kt, :].opt(), gtmp[:], po[:])

            # ============ phase B: theta + attention + wz ================
            z = pers.tile([128, NCT, L], FP32, tag="z")
            zsums = pers.tile([128, NCT, NQB], FP32, tag="zsums")
            zsums2 = pers.tile([128, NCT, NQB], FP32, tag="zsums2")

            for qb in range(NQB):
                qs = qb * QB
                # theta for this query block (re-stream x chunk)
                xc = xchunk.tile([128, NCT, QB], FP32R, tag="xc")
                _load_x_chunk(nc, stage, xc, x_d, qs)
                th = thpool.tile([128, NIT, QB], FP32R, tag="th")
                for it in range(NIT):
                    pt = ps_ft.tile([128, QB], FP32, tag="ft")
                    for ct in range(NCT):
                        nc.tensor.matmul(
                            pt[:], thw[:, ct, it * 128:(it + 1) * 128].opt(),
                            xc[:, ct, :].opt(),
                            start=(ct == 0), stop=(ct == NCT - 1))
                    nc.scalar.activation(th[:, it, :].opt(), pt[:],
                                         ID, bias=tb[:, it:it + 1], scale=1.0)

                y0 = ps_y.tile([128, QB], FP32, tag="y")
                y1 = ps_y.tile([128, QB], FP32, tag="y")
                yps = (y0, y1)
                s_ps = ps_s.tile([1, QB], FP32, tag="s")
                # software-pipelined: mm1(kt) runs ahead of mm2(kt-1)
                pend = None
                for kt in range(NKT):
                    ft = ps_ft.tile([128, QB], FP32, tag="ft")
                    for it in range(NIT):
                        nc.tensor.matmul(
                            ft[:], phi[:, it, kt * 128:(kt + 1) * 128].opt(),
                            th[:, it, :].opt(),
                            start=(it == 0), stop=(it == NIT - 1),
                            skip_group_check=True)
                    e_t = epool.tile([128, QB], FP32R, tag="e")
                    nc.scalar.activation(e_t[:], ft[:], EXP, bias=0.0, scale=1.0)
                    if pend is not None:
                        _emit_mm2(nc, gT, ones_r, yps, s_ps, *pend, NKT)
                    pend = (kt, e_t)
                _emit_mm2(nc, gT, ones_r, yps, s_ps, *pend, NKT)

                # normalize: recip of sums, broadcast via K=1 matmul
                rec = small.tile([1, QB], FP32, tag="rec")
                nc.vector.reciprocal(rec[:], s_ps[:])
                rec_r = small.tile([1, QB], FP32R, tag="rec_r")
                nc.vector.tensor_copy(rec_r[:], rec[:])
                bc_ps = ps_bc.tile([128, QB], FP32, tag="bc")
                nc.tensor.matmul(bc_ps[:], ones1_r[:], rec_r[:],
                                 start=True, stop=True, skip_group_check=True)
                bc = small.tile([128, QB], FP32, tag="bcsb")
                nc.vector.tensor_copy(bc[:], bc_ps[:])
                yt = ypool.tile([128, NIT, QB], FP32R, tag="yt")
                nc.vector.tensor_mul(yt[:, 0, :].opt(), y0[:], bc[:])
                nc.vector.tensor_mul(yt[:, 1, :].opt(), y1[:], bc[:])

                # wz: z[c, q] = wzw^T y  (+ folded bias), + blockwise stats
                for cc in range(NCT):
                    zp = ps_z.tile([128, QB], FP32, tag="z")
                    for it in range(NIT):
                        nc.tensor.matmul(
                            zp[:], wzw[:, it, cc * 128:(cc + 1) * 128].opt(),
                            yt[:, it, :].opt(),
                            start=(it == 0), stop=(it == NIT - 1),
                            skip_group_check=True)
                    zslice = z[:, cc, qs:qs + QB].opt()
                    nc.scalar.activation(
                        zslice, zp[:], ID, bias=wzb[:, cc:cc + 1], scale=1.0,
                        accum_out=zsums[:, cc, qb:qb + 1].opt())
                    scr = small.tile([128, QB], FP32, tag="scr")
                    nc.vector.scalar_tensor_tensor(
                        out=scr[:], in0=zslice, scalar=1.0, in1=zslice,
                        op0=mybir.AluOpType.mult, op1=mybir.AluOpType.mult,
                        accum_out=zsums2[:, cc, qb:qb + 1].opt())

            # ============ phase C: BN stats + AllReduce ==================
            stats = pers.tile([128, 2 * NCT], FP32, tag="stats")
            nc.vector.tensor_reduce(stats[:, 0:NCT], zsums[:, :, :],
                                    axis=mybir.AxisListType.X,
                                    op=mybir.AluOpType.add)
            nc.vector.tensor_reduce(stats[:, NCT:2 * NCT], zsums2[:, :, :],
                                    axis=mybir.AxisListType.X,
                                    op=mybir.AluOpType.add)
            st_in = dram.tile([128, 2 * NCT], FP32)
            st_out = dram.tile([128, 2 * NCT], FP32)
            nc.sync.dma_start(st_in[:], stats[:])
            nc.gpsimd.collective_compute(
                "AllReduce", mybir.AluOpType.add,
                replica_groups=[list(range(N_CORES))],
                ins=[st_in.opt()], outs=[st_out.opt()])
            st_ar = pers.tile([128, 2 * NCT], FP32, tag="st_ar")
            nc.sync.dma_start(st_ar[:], st_out[:])

            inv_n = 1.0 / float(B * L)
            mean = pers.tile([128, NCT], FP32, tag="mean")
            nc.scalar.mul(mean[:], st_ar[:, 0:NCT], inv_n)
            ex2 = pers.tile([128, NCT], FP32, tag="ex2")
            nc.scalar.mul(ex2[:], st_ar[:, NCT:2 * NCT], inv_n)
            var = pers.tile([128, NCT], FP32, tag="var")
            nc.vector.scalar_tensor_tensor(
                out=var[:], in0=mean[:], scalar=-1.0, in1=mean[:],
                op0=mybir.AluOpType.mult, op1=mybir.AluOpType.mult)
            nc.vector.tensor_add(var[:], var[:], ex2[:])   # E[z^2] - mean^2
            nc.vector.tensor_scalar_add(var[:], var[:], 1e-5)
            rinv = pers.tile([128, NCT], FP32, tag="rinv")
            nc.vector.reciprocal(rinv[:], var[:])
            rstd = pers.tile([128, NCT], FP32, tag="rstd")
            nc.scalar.sqrt(rstd[:], rinv[:])
            scl = pers.tile([128, NCT], FP32, tag="scl")
            nc.vector.tensor_mul(scl[:], gam[:], rstd[:])
            shf = pers.tile([128, NCT], FP32, tag="shf")
            nc.vector.tensor_mul(shf[:], mean[:], scl[:])
            nc.vector.tensor_sub(shf[:], bet[:], shf[:])

            # ============ phase D: BN apply + residual ===================
            HL = L // 4
            for ct in range(NCT):
                for h in range(4):
                    hs = h * HL
                    xf2 = stage2.tile([128, HL], FP32, tag="xf2")
                    nc.sync.dma_start(
                        xf2[:], x_d[ct * 128:(ct + 1) * 128, hs:hs + HL])
                    zsl = z[:, ct, hs:hs + HL].opt()
                    nc.scalar.activation(zsl, zsl, ID,
                                         bias=shf[:, ct:ct + 1],
                                         scale=scl[:, ct:ct + 1])
                    nc.vector.tensor_add(xf2[:], xf2[:], zsl)
                    nc.sync.dma_start(
                        out_d[ct * 128:(ct + 1) * 128, hs:hs + HL], xf2[:])
    nc.compile()
    return nc


def _get_nc():
    if "nc" not in _CACHE:
        _CACHE["nc"] = build_nc()
    return _CACHE["nc"]


def _in_maps(x, theta_w, theta_b, phi_w, phi_b, g_w, g_b, wz_w, wz_b,
             bn_gamma, bn_beta):
    f32 = np.float32
    shared = {
        "thw_t": np.ascontiguousarray(theta_w.T, dtype=f32),
        "phw_t": np.ascontiguousarray(phi_w.T, dtype=f32),
        "gw_t": np.ascontiguousarray(g_w.T, dtype=f32),
        "wzw_t": np.ascontiguousarray(wz_w.T, dtype=f32),
        "tb": np.ascontiguousarray(theta_b.reshape(CI, 1), dtype=f32),
        # fold g bias through wz (exact: softmax rows sum to 1)
        "wzb": np.ascontiguousarray(
            (wz_b + wz_w @ g_b).reshape(C, 1), dtype=f32),
        "gam": np.ascontiguousarray(bn_gamma.reshape(C, 1), dtype=f32),
        "bet": np.ascontiguousarray(bn_beta.reshape(C, 1), dtype=f32),
    }
    return [
        {"x": np.ascontiguousarray(x[b], dtype=f32), **shared}
        for b in range(N_CORES)
    ]


def kernel(x, theta_w, theta_b, phi_w, phi_b, g_w, g_b, wz_w, wz_b,
           bn_gamma, bn_beta, _trace=False):
    nc = _get_nc()
    in_maps = _in_maps(np.asarray(x), np.asarray(theta_w), np.asarray(theta_b),
                       np.asarray(phi_w), np.asarray(phi_b), np.asarray(g_w),
                       np.asarray(g_b), np.asarray(wz_w), np.asarray(wz_b),
                       np.asarray(bn_gamma), np.asarray(bn_beta))
    res = run_bass_kernel_spmd(nc, in_maps, list(range(N_CORES)),
                               trace=_trace)
    out = np.stack([res.results[b]["out"] for b in range(N_CORES)], axis=0)
    if _trace:
        kernel.last_results = res
    return out
